# revision 50
# baseline (speedup 1.0000x reference)
"""DiscreteOptionActor Trainium2 kernel.

Computes, for each sample b, logits = MLP_{option[b]}(obs[b]) where each of
the 16 options has its own 3-layer MLP (128 -> 256 -> 256 -> 18, ReLU).

Strategy (MoE routing, expert-sharded, all fp16 on device):
  - Host groups samples by option (argsort). Each core gets two option slots:
    slot 0 one of the 8 smallest groups, slot 1 one of the 8 largest (so the
    kernel ends on slot 1's short tail pair), each padded to a runtime-sized
    per-slot pad (roundup-128 of the slot max, capped with host fallback),
    stored transposed (feature-major [128, pad]) in fp16.
  - Weights packed into two byte tensors per option (w1+b1 gates L1; w2+w3+b2
    follows), one DMA + one semaphore each. Input DMAs are spread over the
    three issue queues (sync/scalar HWDGE + gpsimd SWDGE) in need-time order;
    per-DMA pipeline latency (~2us) dominates, so few well-ordered transfers.
  - Ungated PE warmup matmuls from program entry ramp the HAM clock gate
    (1.2 -> 2.4 GHz) while the first transfers land; the first L1 pair is
    split h-major so its leading matmuls need only the first 512-col chunk.
  - L1/L2 run through a 3-slot PSUM rotation; every 1024-col bias+ReLU drain
    is split into two 512-col halves executed concurrently on ScalarE and
    VectorE, halving the fill->drain->fill latency in the pipeline.
  - L3 (M=18, K=256) packs eight 512-col blocks into the four PE column-group
    quadrants x two column halves of one dedicated PSUM tile (2-way
    concurrent matmuls via tile_position), drained as two [128,512] copies
    (garbage rows included) and DMA'd out packed; the host unpacks. This
    halves L3 matmul time and quarters its drain load.
  - Output-completion waits are elided: the walrus epilogue (~8us semaphore
    reset tail) plus its DMA drains cover the last small transfers.
  - Host scatters results back to original row order and adds b3.
"""

import numpy as np

B, OBS, OPT, H1, H2, A = 65536, 128, 16, 256, 256, 18
NCORES = 8
OPC = OPT // NCORES  # options per core = 2
PAD_CAP = 4992  # beyond this, overflow rows are computed on host

_CACHE = {}


def _mk_pairs(pad):
    out = []
    st = 0
    while st < pad:
        nb = min(1024, pad - st)
        out.append((st, nb))
        st += nb
    return out


def _mk_chunks(pad):
    # 512-col lead chunks for an early L1 start, then 1024s
    sizes = [512, 512, 1024]
    out = []
    st = 0
    for s in sizes:
        if st >= pad:
            break
        nb = min(s, pad - st)
        out.append((st, nb))
        st += nb
    if st < pad:
        # remainder in two roughly equal 1024-aligned chunks
        rem = pad - st
        if rem > 1536:
            first = (rem // 2 + 127) // 128 * 128
            out.append((st, first))
            out.append((st + first, rem - first))
        else:
            out.append((st, rem))
    return out
WARMUPS = [512] * 7 + [128] * 4
ODWAIT = False
WARM_N = 512

# packed weight layouts (bytes per partition):
# w1b: w1 fp16 [256] bytes 0:512 ; b1 f32 [2] bytes 512:520
W1B_BYTES = 520
# w23: w2 fp16 [2,256] bytes 0:1024 ; w3 fp16 [2,18] bytes 1024:1096 ;
#      b2 f32 [2] bytes 1096:1104
W23_BYTES = 1104


def _halves(nb):
    out = []
    h = 0
    while h < nb:
        w = min(512, nb - h)
        out.append((h, w))
        h += w
    return out


def _chunk_hi(chunks, st, nb):
    """Index of the last chunk overlapping [st, st+nb)."""
    hi = 0
    for ci, (cst, cnb) in enumerate(chunks):
        if cst < st + nb:
            hi = ci
    return hi


def _build_v3(pads):
    import concourse.bass as bass
    import concourse.bacc as bacc
    import concourse.mybir as mybir

    f32 = mybir.dt.float32
    f16 = mybir.dt.float16
    AF = mybir.ActivationFunctionType
    ALU = mybir.AluOpType

    PAIRS_O = {o: _mk_pairs(pads[o]) for o in range(OPC)}
    XCHUNKS_O = {o: _mk_chunks(pads[o]) for o in range(OPC)}

    nc = bacc.Bacc(None, target_bir_lowering=False, debug=False)
    xt = [nc.declare_dram_parameter(f"xt{o}", [OBS, pads[o]], f16, isOutput=False)
          for o in range(OPC)]
    w1b = nc.declare_dram_parameter("w1b", [OPC, 128, W1B_BYTES], mybir.dt.uint8,
                                    isOutput=False)
    w23 = nc.declare_dram_parameter("w23", [OPC, 128, W23_BYTES], mybir.dt.uint8,
                                    isOutput=False)
    out = [nc.declare_dram_parameter(f"out{o}", [A, pads[o]], f16, isOutput=True)
           for o in range(OPC)]
    # packed L3 output: psum quadrant layout [128, 1024] per option; block b
    # (512 logits cols) sits at rows 32*(b%4)..+18, cols 512*(b//4)..+512
    outp = [nc.declare_dram_parameter(f"outp{o}", [128, 1024], f16, isOutput=True)
            for o in range(OPC)]

    # --- on-chip tensors ---
    xts = [nc.alloc_sbuf_tensor(f"xts{o}", [OBS, pads[o]], f16) for o in range(OPC)]
    w1s = [nc.alloc_sbuf_tensor(f"w1s{o}", [128, W1B_BYTES], mybir.dt.uint8)
           for o in range(OPC)]
    w23s = [nc.alloc_sbuf_tensor(f"w23s{o}", [128, W23_BYTES], mybir.dt.uint8)
            for o in range(OPC)]
    h1s = [[nc.alloc_sbuf_tensor(f"h1_{o}_{c}", [128, pads[o]], f16) for c in range(2)]
           for o in range(OPC)]
    h2s = [[nc.alloc_sbuf_tensor(f"h2_{o}_{m}", [128, pads[o]], f16) for m in range(2)]
           for o in range(OPC)]
    osbs = [nc.alloc_sbuf_tensor(f"osb{o}", [A, pads[o]], f16) for o in range(OPC)]
    osbp = [nc.alloc_sbuf_tensor(f"osbp{o}", [128, 1024], f16) for o in range(OPC)]
    dummy = nc.alloc_sbuf_tensor("warm_dummy", [128, WARM_N], f16)
    dummy_o = nc.alloc_sbuf_tensor("warm_dummy_o", [128, 1], f32)

    pss = [nc.alloc_psum_tensor(f"ps{s}", [128, 1024], f32) for s in range(3)]
    ps_l3 = nc.alloc_psum_tensor("ps_l3", [128, 1024], f32)

    def w1_ap(o, c):
        return w1s[o].ap()[:, 0:512].bitcast(f16)[:, c * 128:(c + 1) * 128]

    def b1_ap(o, c):
        return w1s[o].ap()[:, 512:520].bitcast(f32)[:, c:c + 1]

    def w2_ap(o, k, m):
        base = k * 256 + m * 128
        return w23s[o].ap()[:, 0:1024].bitcast(f16)[:, base:base + 128]

    def w3_ap(o, k):
        return w23s[o].ap()[:, 1024:1096].bitcast(f16)[:, k * 18:(k + 1) * 18]

    def b2_ap(o, m):
        return w23s[o].ap()[:, 1096:1104].bitcast(f32)[:, m:m + 1]

    # --- semaphores ---
    ws = nc.alloc_semaphore("warm_sem")
    xsem = [[nc.alloc_semaphore(f"x{o}_{ci}") for ci in range(len(XCHUNKS_O[o]))]
            for o in range(OPC)]
    w1sem = [nc.alloc_semaphore(f"w1t{o}") for o in range(OPC)]
    w23sem = [nc.alloc_semaphore(f"w23t{o}") for o in range(OPC)]
    fd = [nc.alloc_semaphore(f"fd{s}") for s in range(3)]
    l3s = [nc.alloc_semaphore(f"l3s{o}") for o in range(OPC)]
    prog = {}
    for o in range(OPC):
        for key in ("h1a", "h1v", "h2a", "h2v", "oa", "ov"):
            prog[(key, o)] = nc.alloc_semaphore(f"{key}{o}")
    odsem = nc.alloc_semaphore("od")

    # --- static schedule containers ---
    pe_ops = []
    act_ops = []
    dve_ops = []
    sync_ops = []
    gps_ops = []

    fill_count = [0, 0, 0]
    slot_prev_drain = [None, None, None]
    fill_idx = 0
    prog_count = {k: 0 for k in prog}
    od_count = [0]

    pe_last_wait = {}

    def pe_wait(waits, sem, val):
        key = sem.name if hasattr(sem, "name") else id(sem)
        if pe_last_wait.get(key, -1) < val:
            waits.append((sem, val))
            pe_last_wait[key] = val

    def emit_fill(data_waits, mms, out_part, drains):
        """mms: (h, w, lhs_fn, rhs_fn, start, stop, mm_waits, tile_pos, ps_lo)
        drains: list of (engine, kind, dst_fn, bias_fn, psem_key, src_lo, src_hi, h0, w0)
        """
        nonlocal fill_idx
        s = fill_idx % 3
        fill_idx += 1
        waits = []
        if slot_prev_drain[s] is not None:
            for sem, cnt in slot_prev_drain[s]:
                pe_wait(waits, sem, cnt)
        for sem, val in data_waits:
            pe_wait(waits, sem, val)
        pe_ops.append((waits, s, mms, out_part, fd[s]))
        fill_count[s] += 1
        fd_thresh = fill_count[s]
        newprev = []
        for (drain_engine, kind, dst_fn, bias_fn, psem_key, src_lo, src_hi,
             h0, w0) in drains:
            sem = prog[psem_key]
            prog_count[psem_key] += 1
            cnt = prog_count[psem_key]
            op = ([(fd[s], fd_thresh)], kind, s, dst_fn, bias_fn, sem,
                  src_lo, src_hi, h0, w0)
            if drain_engine == "act":
                act_ops.append(op)
            else:
                dve_ops.append(op)
            newprev.append((sem, cnt))
        slot_prev_drain[s] = newprev
        return {k: prog_count[k] for k in prog}

    # --- input DMA schedule ---
    def xdma(o, ci):
        cst, cnb = XCHUNKS_O[o][ci]
        return ("dma", [],
                (lambda o=o, cst=cst, cnb=cnb: xts[o].ap()[:, cst:cst + cnb]),
                (lambda o=o, cst=cst, cnb=cnb: xt[o][:, cst:cst + cnb]),
                xsem[o][ci], 16)

    def wdma(o, which):
        if which == 0:
            return ("dma", [], (lambda o=o: w1s[o].ap()[:]),
                    (lambda o=o: w1b[o]), w1sem[o], 16)
        return ("dma", [], (lambda o=o: w23s[o].ap()[:]),
                (lambda o=o: w23[o]), w23sem[o], 16)

    # Three parallel issue queues, ordered by need-time. Per-DMA pipeline
    # latency ~2us dominates, so few DMAs and priority order decide stalls.
    scalar_ops = [xdma(0, 0)]
    sync_ops.extend([wdma(0, 0), wdma(0, 1), wdma(1, 0), wdma(1, 1)])
    gps_ops.extend([xdma(0, 1), xdma(0, 2), xdma(0, 3), xdma(0, 4),
                    xdma(1, 0), xdma(1, 1), xdma(1, 2), xdma(1, 3),
                    xdma(1, 4)])

    di = 0
    eng_load = {"act": 0.0, "dve": 0.0}

    def pick_eng(cols, act_cost=1.07, dve_cost=1.31):
        e = "act" if eng_load["act"] * 1.0 <= eng_load["dve"] else "dve"
        eng_load[e] += cols * (act_cost if e == "act" else dve_cost)
        return e

    l1_thr = {}
    l2_thr = {}

    def emit_l1_split(o, p):
        # first pair only: four single-MM fills ordered h-major, so the
        # leading two matmuls depend only on the first xt chunk
        nonlocal di
        st, nb = PAIRS_O[o][p]
        pc = None
        for h, w in _halves(nb):
            for c in range(2):
                hi = _chunk_hi(XCHUNKS_O[o], st + h, w)
                mms = [(
                    h, w,
                    (lambda o=o, c=c: w1_ap(o, c)),
                    (lambda o=o, st=st, h=h, w=w: xts[o].ap()[:, st + h:st + h + w]),
                    True, True, [(xsem[o][hi], 16)], None, 0,
                )]
                eng = "act" if di % 2 == 0 else "dve"
                drains = [(eng, "relu",
                           (lambda o=o, c=c, st=st, h=h, w=w: h1s[o][c].ap()[:, st + h:st + h + w]),
                           (lambda o=o, c=c: b1_ap(o, c)),
                           ("h1a" if eng == "act" else "h1v", o), 0, 128, h, w)]
                di += 1
                pc = emit_fill([(w1sem[o], 16)], mms, 128, drains)
        l1_thr[(o, p)] = (pc[("h1a", o)], pc[("h1v", o)])

    def emit_l1(o, p):
        nonlocal di
        st, nb = PAIRS_O[o][p]
        mms = []
        drains = []
        for c in range(2):
            for h, w in _halves(nb):
                hi = _chunk_hi(XCHUNKS_O[o], st + h, w)
                mms.append((
                    h, w,
                    (lambda o=o, c=c: w1_ap(o, c)),
                    (lambda o=o, st=st, h=h, w=w: xts[o].ap()[:, st + h:st + h + w]),
                    True, True, [(xsem[o][hi], 16)], None, 0,
                ))
            if nb == 1024:
                e0 = "act" if di % 2 == 0 else "dve"
                e1 = "dve" if e0 == "act" else "act"
                for eng, hh in ((e0, 0), (e1, 512)):
                    drains.append((eng, "relu",
                                   (lambda o=o, c=c, st=st, hh=hh: h1s[o][c].ap()[:, st + hh:st + hh + 512]),
                                   (lambda o=o, c=c: b1_ap(o, c)),
                                   ("h1a" if eng == "act" else "h1v", o),
                                   0, 128, hh, 512))
            else:
                eng = "act" if di % 2 == 0 else "dve"
                drains.append((eng, "relu",
                               (lambda o=o, c=c, st=st, nb=nb: h1s[o][c].ap()[:, st:st + nb]),
                               (lambda o=o, c=c: b1_ap(o, c)),
                               ("h1a" if eng == "act" else "h1v", o), 0, 128, 0, nb))
            di += 1
            # both c-chunks of one pair share a psum slot? no: separate fills
            pc = emit_fill([(w1sem[o], 16)], mms, 128, drains)
            mms = []
            drains = []
        l1_thr[(o, p)] = (pc[("h1a", o)], pc[("h1v", o)])

    def emit_l2(o, p):
        nonlocal di
        st, nb = PAIRS_O[o][p]
        na, nv = l1_thr[(o, p)]
        pc = None
        for m in range(2):
            data_waits = [(w23sem[o], 16)]
            if na:
                data_waits.append((prog[("h1a", o)], na))
            if nv:
                data_waits.append((prog[("h1v", o)], nv))
            mms = []
            for h, w in _halves(nb):
                for k in range(2):
                    mms.append((
                        h, w,
                        (lambda o=o, k=k, m=m: w2_ap(o, k, m)),
                        (lambda o=o, k=k, st=st, h=h, w=w: h1s[o][k].ap()[:, st + h:st + h + w]),
                        k == 0, k == 1, None, None, 0,
                    ))
            if nb == 1024:
                e0 = "act" if di % 2 == 0 else "dve"
                e1 = "dve" if e0 == "act" else "act"
                drains = []
                for eng, hh in ((e0, 0), (e1, 512)):
                    drains.append((eng, "relu",
                                   (lambda o=o, m=m, st=st, hh=hh: h2s[o][m].ap()[:, st + hh:st + hh + 512]),
                                   (lambda o=o, m=m: b2_ap(o, m)),
                                   ("h2a" if eng == "act" else "h2v", o),
                                   0, 128, hh, 512))
            else:
                eng = "act" if di % 2 == 0 else "dve"
                drains = [(eng, "relu",
                           (lambda o=o, m=m, st=st, nb=nb: h2s[o][m].ap()[:, st:st + nb]),
                           (lambda o=o, m=m: b2_ap(o, m)),
                           ("h2a" if eng == "act" else "h2v", o), 0, 128, 0, nb)]
            di += 1
            pc = emit_fill(data_waits, mms, 128, drains)
        l2_thr[(o, p)] = (pc[("h2a", o)], pc[("h2v", o)])

    l3_state = {"war": {}, "full": {0: 0, 1: 0}}

    def emit_l3(o, p):
        nonlocal di
        st, nb = PAIRS_O[o][p]
        na, nv = l2_thr[(o, p)]
        data_waits = [(w23sem[o], 16)]
        if na:
            data_waits.append((prog[("h2a", o)], na))
        if nv:
            data_waits.append((prog[("h2v", o)], nv))
        n_full = sum(1 for (s2, n2) in PAIRS_O[o] if n2 == 1024 and s2 + n2 <= 4096)
        if nb == 1024 and st + nb <= 4096:
            # packed path: 2 blocks -> two psum quadrants of the dedicated
            # L3 tile, matmuls run 2-way col-group concurrent. The tile is
            # drained in [128,512] halves (after packed pairs 2 and 4) so the
            # next option can reuse each half early.
            ch_now = (st // 512) // 4
            waits = []
            if ch_now in l3_state["war"] and l3_state["full"][o] in (0, 2):
                pe_wait(waits, *l3_state["war"][ch_now])
            for sem, val in data_waits:
                pe_wait(waits, sem, val)
            mms = []
            for bi in range(2):
                b = st // 512 + bi
                q, ch = b % 4, b // 4
                for k in range(2):
                    mms.append((
                        ch * 512, 512,
                        (lambda o=o, k=k: w3_ap(o, k)),
                        (lambda o=o, k=k, st=st, bi=bi: h2s[o][k].ap()[:, st + bi * 512:st + bi * 512 + 512]),
                        k == 0, k == 1, None, (0, 32 * q), 32 * q,
                    ))
            pe_ops.append((waits, -1, mms, A, l3s[o]))
            l3_state["full"][o] += 1
            nf = l3_state["full"][o]
            eng = "act" if o == 0 else "dve"
            key = ("oa" if eng == "act" else "ov", o)
            if nf == min(2, n_full) or nf == n_full:
                ch = 0 if nf <= 2 else 1
                c0 = ch * 512
                prog_count[key] += 1
                cnt = prog_count[key]
                op = ([(l3s[o], nf)], "copy", -1,
                      (lambda o=o, c0=c0: osbp[o].ap()[:, c0:c0 + 512]), None,
                      prog[key], 0, 128, c0, 512)
                (act_ops if eng == "act" else dve_ops).append(op)
                l3_state["war"][ch] = (prog[key], cnt)
                od_count[0] += 1
                sync_ops.append(("dma", [(prog[key], cnt)],
                                 (lambda o=o, c0=c0: outp[o][:, c0:c0 + 512]),
                                 (lambda o=o, c0=c0: osbp[o].ap()[:, c0:c0 + 512]),
                                 odsem, 16))
            return
        # tail path: normal rotation fill + [A, nb] drain + DMA
        mms = []
        for h, w in _halves(nb):
            for k in range(2):
                mms.append((
                    h, w,
                    (lambda o=o, k=k: w3_ap(o, k)),
                    (lambda o=o, k=k, st=st, h=h, w=w: h2s[o][k].ap()[:, st + h:st + h + w]),
                    k == 0, k == 1, None, None, 0,
                ))
        eng = "act" if (nb < 1024 or di % 2 == 0) else "dve"
        drains = [(eng, "copy",
                   (lambda o=o, st=st, nb=nb: osbs[o].ap()[:, st:st + nb]),
                   None,
                   ("oa" if eng == "act" else "ov", o), 0, A, 0, nb)]
        di += 1
        pc = emit_fill(data_waits, mms, A, drains)
        dma_waits = [(prog[("oa" if eng == "act" else "ov", o)],
                      pc[("oa" if eng == "act" else "ov", o)])]
        od_count[0] += 1
        dma_op = ("dma", dma_waits,
                  (lambda o=o, st=st, nb=nb: out[o][:, st:st + nb]),
                  (lambda o=o, st=st, nb=nb: osbs[o].ap()[:, st:st + nb]),
                  odsem, 16)
        sync_ops.append(dma_op)

    # global software pipeline: L1 two pair-groups ahead; L3 trails by one
    l1q = [(o, p) for o in range(OPC) for p in range(len(PAIRS_O[o]))]
    l2q = list(l1q)
    l3q = []
    emit_l1_split(*l1q.pop(0))
    emit_l1(*l1q.pop(0))
    for (o, p) in l2q:
        emit_l2(o, p)
        if l1q:
            emit_l1(*l1q.pop(0))
        l3q.append((o, p))
        if len(l3q) > 1:
            emit_l3(*l3q.pop(0))
    while l3q:
        emit_l3(*l3q.pop(0))

    n_od = od_count[0]

    # --- emit engine programs ---
    with nc.Block(no_gpsimd_drain=True) as block:

        @block.gpsimd
        def _(eng):
            for op in gps_ops:
                kind, waits, dst_fn, src_fn, sem, val = op
                for wsem_, wval in waits:
                    eng.wait_ge(wsem_, wval)
                eng.dma_start(out=dst_fn(), in_=src_fn()).then_inc(sem, val)

        @block.sync
        def _(eng):
            for op in sync_ops:
                kind, waits, dst_fn, src_fn, sem, val = op
                for wsem_, wval in waits:
                    eng.wait_ge(wsem_, wval)
                eng.dma_start(out=dst_fn(), in_=src_fn()).then_inc(sem, val)
            if ODWAIT:
                eng.wait_ge(odsem, 16 * n_od)

        @block.tensor
        def _(eng):
            for wn in WARMUPS:
                nc.tensor.matmul(
                    pss[0].ap()[:128, :wn], dummy.ap()[:, :128], dummy.ap()[:, :wn],
                    start=True, stop=True,
                )
            mm_seen = {}
            for waits, s, mms, out_part, fdsem in pe_ops:
                for wsem_, wval in waits:
                    eng.wait_ge(wsem_, wval)
                for j, (h, w, lhs_fn, rhs_fn, stt, stp, mwaits, tp, plo) in enumerate(mms):
                    if mwaits:
                        for wsem_, wval in mwaits:
                            key = wsem_.name if hasattr(wsem_, "name") else id(wsem_)
                            if mm_seen.get(key, -1) < wval:
                                eng.wait_ge(wsem_, wval)
                                mm_seen[key] = wval
                    kw = {}
                    if tp is not None:
                        kw["tile_position"] = tp
                    pst = ps_l3 if s == -1 else pss[s]
                    inst = nc.tensor.matmul(
                        pst.ap()[plo:plo + out_part, h:h + w],
                        lhs_fn(), rhs_fn(), start=stt, stop=stp, **kw,
                    )
                    if j == len(mms) - 1:
                        inst.then_inc(fdsem, 1)

        @block.scalar
        def _(eng):
            for op in scalar_ops:
                kind, waits, dst_fn, src_fn, sem, val = op
                eng.dma_start(out=dst_fn(), in_=src_fn()).then_inc(sem, val)
            nc.scalar.activation(dummy_o.ap()[:], dummy.ap()[:, 0:1], AF.Relu, bias=0.0)
            for waits, kind, s, dst_fn, bias_fn, sem, lo, hi, h0, w0 in act_ops:
                for wsem_, wval in waits:
                    eng.wait_ge(wsem_, wval)
                dst = dst_fn()
                src = (ps_l3 if s == -1 else pss[s]).ap()[lo:hi, h0:h0 + w0]
                if kind == "relu":
                    inst = nc.scalar.activation(dst, src, AF.Relu, bias=bias_fn())
                else:
                    inst = nc.scalar.activation(dst, src, AF.Copy)
                inst.then_inc(sem, 1)

        @block.vector
        def _(eng):
            for waits, kind, s, dst_fn, bias_fn, sem, lo, hi, h0, w0 in dve_ops:
                for wsem_, wval in waits:
                    eng.wait_ge(wsem_, wval)
                dst = dst_fn()
                src = (ps_l3 if s == -1 else pss[s]).ap()[lo:hi, h0:h0 + w0]
                if kind == "relu":
                    inst = nc.vector.tensor_scalar(
                        dst, src, bias_fn(), 0.0, ALU.add, ALU.max
                    )
                else:
                    inst = nc.vector.tensor_copy(dst, src)
                inst.then_inc(sem, 1)

    nc.compile()
    return nc


def _get_program(pads):
    key = tuple(pads)
    if key not in _CACHE:
        _CACHE[key] = _build_v3(pads)
    return _CACHE[key]


def _prep(inputs):
    obs = np.ascontiguousarray(np.asarray(inputs["obs"], dtype=np.float32))
    option = np.asarray(inputs["option"]).astype(np.int64, copy=False)
    W1 = np.asarray(inputs["W1"], dtype=np.float32)
    b1 = np.asarray(inputs["b1"], dtype=np.float32)
    W2 = np.asarray(inputs["W2"], dtype=np.float32)
    b2 = np.asarray(inputs["b2"], dtype=np.float32)
    W3 = np.asarray(inputs["W3"], dtype=np.float32)
    b3 = np.asarray(inputs["b3"], dtype=np.float32)

    order = np.argsort(option, kind="stable")
    sorted_opt = option[order]
    starts = np.searchsorted(sorted_opt, np.arange(OPT + 1))
    idx_per_opt = [order[starts[o]: starts[o + 1]] for o in range(OPT)]
    counts = np.array([len(ix) for ix in idx_per_opt])

    # slot 0 <- the 8 largest options (one per core), slot 1 <- the 8 smallest;
    # per-slot pad = roundup(max count in slot, 128), capped (host computes
    # overflow rows beyond the cap)
    by_size = np.argsort(-counts, kind="stable")
    # slot 0 (processed first) gets the 8 smallest groups, slot 1 the 8
    # largest: the kernel then ends on slot 1's short tail pair
    slot_opts = [list(by_size[NCORES:]), list(by_size[:NCORES])]
    pads = []
    for s in range(OPC):
        mx = max(counts[o] for o in slot_opts[s])
        pads.append(int(min(-(-max(mx, 128) // 128) * 128, PAD_CAP)))
    pads = tuple(pads)

    def pack_w(o):
        w1p = np.ascontiguousarray(W1[o].astype(np.float16))  # [128, 256]
        b1p = np.ascontiguousarray(b1[o].reshape(2, 128).T.astype(np.float32))
        w1bp = np.concatenate([w1p.view(np.uint8), b1p.view(np.uint8)], axis=1)
        w2p = np.ascontiguousarray(
            W2[o].reshape(2, 128, H2).transpose(1, 0, 2).astype(np.float16)
        ).reshape(128, -1)
        w3p = np.ascontiguousarray(
            W3[o].reshape(2, 128, A).transpose(1, 0, 2).astype(np.float16)
        ).reshape(128, -1)
        b2p = np.ascontiguousarray(b2[o].reshape(2, 128).T.astype(np.float32))
        w23p = np.concatenate(
            [w2p.view(np.uint8), w3p.view(np.uint8), b2p.view(np.uint8)], axis=1)
        return w1bp, w23p

    in_maps = []
    for core in range(NCORES):
        m = {"w1b": np.zeros((OPC, 128, W1B_BYTES), np.uint8),
             "w23": np.zeros((OPC, 128, W23_BYTES), np.uint8)}
        for s in range(OPC):
            o = slot_opts[s][core]
            idx = idx_per_opt[o][:pads[s]]
            xtc = np.zeros((OBS, pads[s]), np.float16)
            xtc[:, : len(idx)] = obs[idx].T
            m[f"xt{s}"] = xtc
            m["w1b"][s], m["w23"][s] = pack_w(o)
        in_maps.append(m)
    host = dict(obs=obs, W1=W1, b1=b1, W2=W2, b2=b2, W3=W3, b3=b3)
    return in_maps, idx_per_opt, slot_opts, pads, host


def _unshard(results, idx_per_opt, slot_opts, pads, host):
    out_full = np.empty((B, 1, A), np.float32)
    for core in range(NCORES):
        for s in range(OPC):
            o = slot_opts[s][core]
            resp = results[core][f"outp{s}"]  # [128, 1024] packed quadrants
            rest = results[core][f"out{s}"]   # [A, pads[s]] tail only
            idx = idx_per_opt[o]
            n = min(len(idx), pads[s])
            full_span = min((pads[s] // 1024) * 1024, 4096)
            logits = np.empty((A, pads[s]), np.float32)
            for b in range(full_span // 512):
                q, ch = b % 4, b // 4
                logits[:, b * 512:(b + 1) * 512] = \
                    resp[32 * q:32 * q + A, ch * 512:(ch + 1) * 512]
            if full_span < pads[s]:
                logits[:, full_span:] = rest[:, full_span:]
            out_full[idx[:n], 0, :] = logits[:, :n].T + host["b3"][o]
            if len(idx) > n:  # overflow beyond pad: compute on host
                rows = host["obs"][idx[n:]]
                h = np.maximum(rows @ host["W1"][o] + host["b1"][o], 0.0)
                h = np.maximum(h @ host["W2"][o] + host["b2"][o], 0.0)
                out_full[idx[n:], 0, :] = h @ host["W3"][o] + host["b3"][o]
    return out_full


def run(inputs, trace=False, **spmd_kwargs):
    """Run the kernel; returns (output, BassKernelResults)."""
    from concourse.bass_utils import run_bass_kernel_spmd

    in_maps, idx_per_opt, slot_opts, pads, host = _prep(inputs)
    nc = _get_program(pads)
    try:
        br = run_bass_kernel_spmd(
            nc, in_maps, list(range(NCORES)), trace=trace, **spmd_kwargs
        )
    except Exception:
        _CACHE.clear()
        nc = _get_program(pads)
        br = run_bass_kernel_spmd(
            nc, in_maps, list(range(NCORES)), trace=trace, **spmd_kwargs
        )
    return _unshard(br.results, idx_per_opt, slot_opts, pads, host), br


def kernel(**inputs):
    out, _ = run(inputs)
    return out
